# revision 12
# baseline (speedup 1.0000x reference)
"""Trainium2 Bass kernel for nn_AffineMaskGenerator.

For each pixel p with color x (3-vec from `images`) and shifted color y
(3-vec from `shifted_images`), and each class k:

    log_mask[b, k, h, w] = -||W_k @ x + b_k - y||^2 / (2 sigma^2)

Strategy (pure data parallel over batch, 4 images per NeuronCore):
  - Fold the affine map into one matmul: with s = 1/(sqrt(2)*sigma),
    diff = s*(W_k x - y) + s*b_k is linear in (x, y); the bias enters
    later through the Square activation's per-partition bias.
    MM1: lhsT [30, 120] x rhs [30, 512] -> PSUM [120, 512]; the 120
    rows are 5 pixel-groups x (8 classes x 3 channels) = 5 px/PE-cycle.
    Four MM1s run concurrently in disjoint PE row groups 0/32/64/96
    (tile_position packing, 4 super-tiles per "quad").
  - Square via ScalarE activation(Square, bias=s*b) into bf16; ~1.5 of
    12 square tiles per quad go to VectorE (tensor_scalar add + bf16
    tensor_mul) to balance the engines.  DVE cannot read PSUM twice,
    so plain tensor_mul on PSUM is unavailable.
  - MM2: lhsT [120, 40] of -1 entries sums squares over channels ->
    [40, 512] final values in PSUM (5 groups x 8 classes).  Chunks
    (i, i+3) pack into one PSUM bank at partition offsets 0/64
    (tile_position) so a single DVE copy evacuates both, and each obuf
    half is a contiguous 1536-px run per (group, class) -> one 3-dim
    store DMA per half.
  - Matmuls run in bf16 (this walrus build rejects f32/f32r matmuls;
    PE is also clamped to ~1.2 GHz here, so tile_position concurrency
    is the only matmul-throughput lever).  Inputs are pre-cast to bf16
    on the host and concatenated to one [BL, 6, H, W] tensor: one
    3-dim SWDGE DMA loads a whole super-tile band set.
  - Pixel groups are contiguous 3072-px bands inside each 15360-px
    super-tile; the image tail is covered by an overlapping
    (idempotent) extra tile per image.
"""

import ml_dtypes
import numpy as np

import concourse.bass as bass
import concourse.mybir as mybir
import concourse.tile as tile
from concourse.tile import ScopedClock
from concourse import bass_utils

F32 = mybir.dt.float32
BF16 = mybir.dt.bfloat16

B, C, H, Wd = 32, 3, 512, 512
K = 8
NCORES = 8
BL = B // NCORES            # images per core
PLANE = H * Wd              # 262144 pixels per channel plane

N = 512                     # pixels per chunk (one PSUM bank of f32)
G = 5                       # pixel groups per tile (5 px / PE cycle)
TPS = 6                     # chunks per super-tile
BAND = TPS * N              # 3072 px: one group's contiguous band
SPX = G * BAND              # 15360 pixels per super-tile
NSUP = PLANE // SPX         # 17 full super-tiles per image
OV_BASE = PLANE - G * N     # overlap tile covers the image tail

_patched = False


MAX_WAITS = 1   # this walrus build rejects instructions with more sync waits


def _split_excess_waits(nc):
    """Walrus 'Too many sync wait commands': any instruction carrying
    more than MAX_WAITS sem waits gets the excess moved onto fresh NoOps
    inserted just before it on the same engine (engines execute their
    instruction stream in block order, so semantics are unchanged)."""
    import bass_rust
    counter = [0]
    for f in nc.m.functions:
        for bb in f.blocks:
            new_insts = []
            for inst in bb.instructions:
                si = inst.sync_info
                waits = list(si.on_wait or []) if si is not None else []
                if len(waits) > MAX_WAITS:
                    rest = waits[:-MAX_WAITS]
                    si.on_wait = waits[-MAX_WAITS:]
                    while rest:
                        counter[0] += 1
                        nop = bass_rust.InstNoOp(
                            name=f"waitsplit_{counter[0]}", ins=[], outs=[])
                        nop.engine = inst.engine
                        nop.sync_info = mybir.SyncInfo(
                            on_wait=rest[:MAX_WAITS], on_update=[])
                        rest = rest[MAX_WAITS:]
                        new_insts.append(nop)
                new_insts.append(inst)
            bb.instructions = new_insts


def _patch_tile_drain():
    """Rebuild the kernel-tail drain with split waits + run the global
    excess-wait splitter after Tile lowering."""
    global _patched
    if _patched:
        return
    _patched = True

    def _drain_and_barrier(self, tick_clock, wait_clock):
        drain_inst = self.nc.sync.drain()
        wait_clock.add_sem_waits(
            drain_inst.ins, ScopedClock({None: tick_clock.global_clock})
        )
        si = drain_inst.ins.sync_info
        waits = list(si.on_wait or []) if si is not None else []
        if len(waits) > 1:
            si.on_wait = waits[:1]
            for w in waits[1:]:
                d2 = self.nc.sync.drain()
                d2.ins.sync_info = mybir.SyncInfo(on_wait=[w], on_update=[])
        self.nc.all_engine_barrier()
        popped = self.nc._tile_sem_poison_stack.pop()
        assert popped is self._sem_poison
        self.nc.clear_and_free_semaphores(list(self.sems.allocated().values()))
        self.nc.all_engine_barrier()
        _split_excess_waits(self.nc)

    tile.TileContext._drain_and_barrier = _drain_and_barrier


def _host_weights(Wm, bm, sigma):
    """w1 [31, 120]: row 5c+g = x_c of group g, 15+5o+g = y_o of group g,
    30 = ones; col m = 24g+3k+o.  w2 [120, 40]: channel-sum, col 8g+k."""
    s = 1.0 / (np.sqrt(2.0) * float(sigma))
    w1 = np.zeros((30, 120), np.float32)
    w2 = np.zeros((120, 40), np.float32)
    bias = np.zeros((120, 1), np.float32)
    for g in range(G):
        for k in range(K):
            for o in range(C):
                m = 24 * g + 3 * k + o
                for c in range(C):
                    w1[5 * c + g, m] = s * Wm[k, o, c]
                w1[15 + 5 * o + g, m] = -s
                bias[m, 0] = s * bm[k, o]
                w2[m, 8 * g + k] = -1.0
    return w1, w2, bias


def build_nc():
    _patch_tile_drain()
    nc = bass.Bass("TRN2", target_bir_lowering=False, debug=False)
    # xy: host-side concat of images & shifted along channels, pre-cast
    # to bf16 (halves input DMA bytes, avoids the slow SWDGE cast path)
    xy = nc.dram_tensor("xy", [BL, 2 * C, H, Wd], BF16, kind="ExternalInput")
    w1 = nc.dram_tensor("w1", [30, 120], BF16, kind="ExternalInput")
    w2 = nc.dram_tensor("w2", [120, 40], BF16, kind="ExternalInput")
    bias = nc.dram_tensor("bias", [120, 1], F32, kind="ExternalInput")
    # Output in bf16 (halves store bytes; rel-err budget is 2e-2) and in a
    # HW-friendly scrambled layout: each half-obuf stores as one fully
    # contiguous 40x1536 run, so the store AP's outer dim is 40 partition
    # rows -> HWDGE sprays across all 16 SDMA engines instead of 5.  The
    # host unscrambles (postprocess).
    out = nc.dram_tensor("out", [BL, NSUP, 2, 40, 3 * N], BF16,
                         kind="ExternalOutput")
    out_tail = nc.dram_tensor("out_tail", [BL, 40, N], BF16,
                              kind="ExternalOutput")

    from contextlib import ExitStack
    with tile.TileContext(nc, pool_alloc_mode="queue") as tc, ExitStack() as ctx:
        singles = ctx.enter_context(tc.tile_pool(name="singles", bufs=1))
        chan_pool = ctx.enter_context(tc.tile_pool(name="chan", bufs=3))
        sq_pool = ctx.enter_context(tc.tile_pool(name="sq", bufs=30))
        obuf_pool = ctx.enter_context(tc.tile_pool(name="obuf", bufs=8))
        tmp_pool = ctx.enter_context(tc.tile_pool(name="tmp", bufs=2))
        pd_pool = ctx.enter_context(tc.tile_pool(name="pd", bufs=2, space="PSUM"))
        po_pool = ctx.enter_context(tc.tile_pool(name="po", bufs=2, space="PSUM"))

        # w1 replicated at partition bands 0/32/64/96 so four MM1s run
        # concurrently in disjoint PE row groups (tile_position packing)
        w1_t = singles.tile([126, 120], BF16)
        for r in range(4):
            nc.gpsimd.dma_start(out=w1_t[32 * r:32 * r + 30, :], in_=w1.ap())
        w2_t = singles.tile([120, 40], BF16)
        nc.gpsimd.dma_start(out=w2_t[:, :], in_=w2.ap())
        # per-partition bias s*b[k,o]: applied inside the Square activation
        # (ScalarE) / via tensor_scalar add (VectorE path)
        bias_t = singles.tile([120, 1], F32)
        nc.gpsimd.dma_start(out=bias_t[:, :], in_=bias.ap())

        def mm1(chan, band, pd, t, j):
            """Chunk j of `chan` band -> pd column t.  Bands use disjoint
            PE row groups, so the four mm1s execute concurrently."""
            nc.tensor.matmul(
                pd[:, bass.ts(t, N)], w1_t[band:band + 30, :],
                chan[band + 0:band + 30, bass.ts(j, N)],
                start=True, stop=True, tile_position=(band, 0))

        def square(pd, n_tiles):
            sq = sq_pool.tile([120, 2 * N], BF16, tag="sq")
            nc.scalar.activation(
                sq[:, 0:n_tiles * N], pd[:, 0:n_tiles * N],
                mybir.ActivationFunctionType.Square,
                bias=bias_t[:, 0:1], scale=1.0)
            return sq

        def square_dve(pd):
            """Square via VectorE: PSUM->SBUF bf16 copy (1x) + bf16
            tensor_mul (2x).  Less efficient than ScalarE but runs on
            the otherwise under-used DVE -- used to offload ScalarE."""
            tmp = tmp_pool.tile([120, 2 * N], BF16, tag="tmp")
            nc.vector.tensor_scalar_add(tmp[:, :], pd[:, 0:2 * N],
                                        bias_t[:, 0:1])
            sq = sq_pool.tile([120, 2 * N], BF16, tag="sq")
            nc.vector.tensor_mul(sq[:, :], tmp[:, :], tmp[:, :])
            return sq

        def emit_po(sq_of, pairs, obuf, col0):
            """MM2s for chunk-pairs [(i, i+3), ...] -> one po tile; pair t
            lands at po cols 512t, halves (i / i+3) at partition 0/64
            (tile_position col packing).  One DVE cast evacuates the whole
            po into obuf columns starting at col0.  With this pairing
            obuf[0:40] holds chunks 0..2 and obuf[64:104] chunks 3..5 --
            each a contiguous 1536-px bf16 run per (g, k)."""
            po = po_pool.tile([104, 2 * N], F32, tag="po")
            for t, i in enumerate(pairs):
                for u, j in enumerate((i, i + 3)):
                    sq, col = sq_of(j)
                    nc.tensor.matmul(
                        po[64 * u:64 * u + 40, bass.ts(t, N)], w2_t[:, :],
                        sq[:, bass.ts(col, N)],
                        start=True, stop=True, tile_position=(0, 64 * u))
            w = len(pairs) * N
            nc.vector.tensor_copy(obuf[:, col0:col0 + w], po[:, 0:w])

        def load_chan(chan, band, s):
            """Load super-tile s (global index) into `chan` band 0/64."""
            img, S = divmod(s, NSUP)
            ib = img * 2 * C * PLANE
            px0 = S * SPX
            src_xy = bass.AP(xy, ib + px0,
                             [[PLANE, 2 * C], [BAND, G], [1, BAND]])
            nc.gpsimd.dma_start(out=chan[band + 0:band + 30], in_=src_xy)

        def store_out(obuf, s):
            img, S = divmod(s, NSUP)
            for half in range(2):
                src = bass.AP(
                    obuf.tensor, obuf.offset + half * 64 * (3 * N),
                    [[3 * N, 40], [1, 3 * N]])
                dst = bass.AP(
                    out, ((img * NSUP + S) * 2 + half) * 40 * (3 * N),
                    [[3 * N, 40], [1, 3 * N]])
                nc.sync.dma_start(out=dst, in_=src)

        def emit_mm2_store(sqs, s0):
            for b in range(4):
                half, bi = b // 2, b % 2
                obuf = obuf_pool.tile([104, 3 * N], BF16, tag="obuf")

                def sq_of(j, half=half, bi=bi):
                    return (sqs[(half, j)], bi)
                emit_po(sq_of, (0, 1), obuf, 0)
                emit_po(sq_of, (2,), obuf, 2 * N)
                store_out(obuf, s0 + b)

        NSUP_CORE = BL * NSUP          # 68 super-tiles per core
        pending = []
        for q in range(NSUP_CORE // 4):
            chan = chan_pool.tile([127, TPS * N], BF16, tag="chan")
            for r in range(4):
                load_chan(chan, 32 * r, 4 * q + r)

            # Drain the 2-quad-delayed MM2/store first: its inputs (squares
            # of quad q-2) are long ready, so these matmuls never stall the
            # in-order PE stream (no head-of-line blocking).
            if len(pending) == 2:
                emit_mm2_store(*pending.pop(0))

            sqs = {}
            for j in range(TPS):
                pd01 = pd_pool.tile([120, 2 * N], F32, tag="pd")
                pd23 = pd_pool.tile([120, 2 * N], F32, tag="pd")
                # all four MM1s adjacent in the PE stream -> 4-way
                # tile_position concurrency
                mm1(chan, 0, pd01, 0, j)
                mm1(chan, 32, pd01, 1, j)
                mm1(chan, 64, pd23, 0, j)
                mm1(chan, 96, pd23, 1, j)
                sqs[(0, j)] = square(pd01, 2)
                if j == 2 or (j == 5 and q % 2 == 1):
                    sqs[(1, j)] = square_dve(pd23)
                else:
                    sqs[(1, j)] = square(pd23, 2)
            pending.append((sqs, 4 * q))
        for p in pending:
            emit_mm2_store(*p)

        # image-tail overlap tiles (re-compute 1536 px each; idempotent).
        for img in range(BL):
            ib = img * 2 * C * PLANE
            chan = chan_pool.tile([95, TPS * N], BF16, tag="chan")
            src_xy = bass.AP(xy, ib + OV_BASE, [[PLANE, 2 * C], [N, G], [1, N]])
            nc.gpsimd.dma_start(out=chan[0:30, 0:N], in_=src_xy)
            obuf = obuf_pool.tile([104, 3 * N], BF16, tag="obuf")
            pd = pd_pool.tile([120, 2 * N], F32, tag="pd")
            mm1(chan, 0, pd, 0, 0)
            sq_ov = square(pd, 1)
            po = po_pool.tile([104, 2 * N], F32, tag="po")
            nc.tensor.matmul(po[0:40, 0:N], w2_t[:, :], sq_ov[:, bass.ts(0, N)],
                             start=True, stop=True, tile_position=(0, 0))
            nc.vector.tensor_copy(obuf[:, bass.ts(0, N)], po[0:104, 0:N])
            dst = bass.AP(out_tail, img * 40 * N, [[N, 40], [1, N]])
            src = bass.AP(obuf.tensor, obuf.offset, [[3 * N, 40], [1, N]])
            nc.sync.dma_start(out=dst, in_=src)

    return nc


def _run(in_maps, trace=False, tmpdir=None):
    nc = build_nc()
    if trace:
        # inject the NTFF profile hook (normally absent in this image)
        import sys, types
        from trn_agent_boot.trn_boot import _ntff_profile_via_ctypes
        hook = _ntff_profile_via_ctypes('/opt/axon/libaxon_pjrt.so')
        m = types.ModuleType("antenv.axon_hooks")
        m.get_axon_ntff_profile_hook = lambda: hook
        m.set_axon_ntff_profile_hook = lambda h: None
        sys.modules["antenv.axon_hooks"] = m
        bass_utils.upload_artifacts = lambda d: d
    return bass_utils.run_bass_kernel_spmd(
        nc, in_maps, core_ids=list(range(NCORES)), trace=trace, tmpdir=tmpdir)


def make_in_maps(images, shifted_images, W, b, sigma):
    w1, w2, bias = _host_weights(np.asarray(W), np.asarray(b), sigma)
    w1 = w1.astype(ml_dtypes.bfloat16)
    w2 = w2.astype(ml_dtypes.bfloat16)
    xy = np.concatenate(
        [np.asarray(images, np.float32), np.asarray(shifted_images, np.float32)],
        axis=1).astype(ml_dtypes.bfloat16)
    in_maps = []
    for i in range(NCORES):
        in_maps.append({
            "xy": np.ascontiguousarray(xy[BL * i:BL * (i + 1)]),
            "w1": w1, "w2": w2, "bias": bias,
        })
    return in_maps


def postprocess(res):
    """Unscramble the HW output layout back to [B, K, H, W] float32.

    main[img, s, h, m=(8g+k), j] -> out[img, k, s*SPX + g*BAND + h*3N + j]
    tail[img, m=(8g+k), j]       -> out[img, k, OV_BASE + g*N + j]
    """
    outs = []
    for i in range(NCORES):
        main = np.asarray(res.results[i]["out"]).astype(np.float32)
        tail = np.asarray(res.results[i]["out_tail"]).astype(np.float32)
        main = main.reshape(BL, NSUP, 2, G, K, 3 * N)
        main = main.transpose(0, 4, 1, 3, 2, 5).reshape(BL, K, NSUP * SPX)
        full = np.empty((BL, K, PLANE), np.float32)
        full[:, :, :NSUP * SPX] = main
        tail = tail.reshape(BL, G, K, N).transpose(0, 2, 1, 3)
        full[:, :, OV_BASE:] = tail.reshape(BL, K, G * N)
        outs.append(full.reshape(BL, K, H, Wd))
    return np.concatenate(outs, axis=0)


def kernel(images, shifted_images, W, b, sigma):
    in_maps = make_in_maps(images, shifted_images, W, b, sigma)
    res = _run(in_maps, trace=False)
    return postprocess(res)



# revision 16
# speedup vs baseline: 1.1867x; 1.1867x over previous
"""Trainium2 Bass kernel for nn_AffineMaskGenerator.

For each pixel p with color x (3-vec from `images`) and shifted color y
(3-vec from `shifted_images`), and each class k:

    log_mask[b, k, h, w] = -||W_k @ x + b_k - y||^2 / (2 sigma^2)

Strategy (pure data parallel over batch, 4 images per NeuronCore):
  - Fold the affine map into one matmul: with s = 1/(sqrt(2)*sigma),
    diff = s*(W_k x - y) + s*b_k is linear in (x, y); the bias enters
    later through the Square activation's per-partition bias.
    MM1: lhsT [30, 120] x rhs [30, 512] -> PSUM [120, 512]; the 120
    rows are 5 pixel-groups x (8 classes x 3 channels) = 5 px/PE-cycle.
    Four MM1s run concurrently in disjoint PE row groups 0/32/64/96
    (tile_position packing, 4 super-tiles per "quad").
  - Square via ScalarE activation(Square, bias=s*b) into bf16; ~1.5 of
    12 square tiles per quad go to VectorE (tensor_scalar add + bf16
    tensor_mul) to balance the engines.  DVE cannot read PSUM twice,
    so plain tensor_mul on PSUM is unavailable.
  - MM2: lhsT [120, 40] of -1 entries sums squares over channels ->
    [40, 512] final values in PSUM (5 groups x 8 classes).  Chunks
    (i, i+3) pack into one PSUM bank at partition offsets 0/64
    (tile_position) so a single DVE copy evacuates both, and each obuf
    half is a contiguous 1536-px run per (group, class) -> one 3-dim
    store DMA per half.
  - Matmuls run in bf16 (this walrus build rejects f32/f32r matmuls;
    PE is also clamped to ~1.2 GHz here, so tile_position concurrency
    is the only matmul-throughput lever).  Inputs are pre-cast to bf16
    on the host and concatenated to one [BL, 6, H, W] tensor: one
    3-dim SWDGE DMA loads a whole super-tile band set.
  - Pixel groups are contiguous 3072-px bands inside each 15360-px
    super-tile; the image tail is covered by an overlapping
    (idempotent) extra tile per image.
"""

import ml_dtypes
import numpy as np

import concourse.bass as bass
import concourse.mybir as mybir
import concourse.tile as tile
from concourse.tile import ScopedClock
from concourse import bass_utils

F32 = mybir.dt.float32
BF16 = mybir.dt.bfloat16

B, C, H, Wd = 32, 3, 512, 512
K = 8
NCORES = 8
BL = B // NCORES            # images per core
PLANE = H * Wd              # 262144 pixels per channel plane

N = 512                     # pixels per chunk (one PSUM bank of f32)
G = 5                       # pixel groups per tile (5 px / PE cycle)
TPS = 6                     # chunks per super-tile
BAND = TPS * N              # 3072 px: one group's contiguous band
SPX = G * BAND              # 15360 pixels per super-tile
NSUP = PLANE // SPX         # 17 full super-tiles per image
OV_BASE = PLANE - G * N     # overlap tile covers the image tail

_patched = False


MAX_WAITS = 1   # this walrus build rejects instructions with more sync waits


def _split_excess_waits(nc):
    """Walrus 'Too many sync wait commands': any instruction carrying
    more than MAX_WAITS sem waits gets the excess moved onto fresh NoOps
    inserted just before it on the same engine (engines execute their
    instruction stream in block order, so semantics are unchanged)."""
    import bass_rust
    counter = [0]
    for f in nc.m.functions:
        for bb in f.blocks:
            new_insts = []
            for inst in bb.instructions:
                si = inst.sync_info
                waits = list(si.on_wait or []) if si is not None else []
                if len(waits) > MAX_WAITS:
                    rest = waits[:-MAX_WAITS]
                    si.on_wait = waits[-MAX_WAITS:]
                    while rest:
                        counter[0] += 1
                        nop = bass_rust.InstNoOp(
                            name=f"waitsplit_{counter[0]}", ins=[], outs=[])
                        nop.engine = inst.engine
                        nop.sync_info = mybir.SyncInfo(
                            on_wait=rest[:MAX_WAITS], on_update=[])
                        rest = rest[MAX_WAITS:]
                        new_insts.append(nop)
                new_insts.append(inst)
            bb.instructions = new_insts


def _patch_tile_drain():
    """Rebuild the kernel-tail drain with split waits + run the global
    excess-wait splitter after Tile lowering."""
    global _patched
    if _patched:
        return
    _patched = True

    def _drain_and_barrier(self, tick_clock, wait_clock):
        drain_inst = self.nc.sync.drain()
        wait_clock.add_sem_waits(
            drain_inst.ins, ScopedClock({None: tick_clock.global_clock})
        )
        si = drain_inst.ins.sync_info
        waits = list(si.on_wait or []) if si is not None else []
        if len(waits) > 1:
            si.on_wait = waits[:1]
            for w in waits[1:]:
                d2 = self.nc.sync.drain()
                d2.ins.sync_info = mybir.SyncInfo(on_wait=[w], on_update=[])
        self.nc.all_engine_barrier()
        popped = self.nc._tile_sem_poison_stack.pop()
        assert popped is self._sem_poison
        self.nc.clear_and_free_semaphores(list(self.sems.allocated().values()))
        self.nc.all_engine_barrier()
        _split_excess_waits(self.nc)

    tile.TileContext._drain_and_barrier = _drain_and_barrier


def _host_weights(Wm, bm, sigma):
    """w1 [31, 120]: row 5c+g = x_c of group g, 15+5o+g = y_o of group g,
    30 = ones; col m = 24g+3k+o.  w2 [120, 40]: channel-sum, col 8g+k."""
    s = 1.0 / (np.sqrt(2.0) * float(sigma))
    w1 = np.zeros((30, 120), np.float32)
    w2 = np.zeros((120, 40), np.float32)
    bias = np.zeros((120, 1), np.float32)
    for g in range(G):
        for k in range(K):
            for o in range(C):
                m = 24 * g + 3 * k + o
                for c in range(C):
                    w1[5 * c + g, m] = s * Wm[k, o, c]
                w1[15 + 5 * o + g, m] = -s
                bias[m, 0] = s * bm[k, o]
                w2[m, 8 * g + k] = -1.0
    return w1, w2, bias


def build_nc():
    _patch_tile_drain()
    nc = bass.Bass("TRN2", target_bir_lowering=False, debug=False)
    # xy: host-side concat of images & shifted along channels, pre-cast
    # to bf16 (halves input DMA bytes, avoids the slow SWDGE cast path)
    xy = nc.dram_tensor("xy", [BL, 2 * C, H, Wd], BF16, kind="ExternalInput")
    w1 = nc.dram_tensor("w1", [30, 120], BF16, kind="ExternalInput")
    w2 = nc.dram_tensor("w2", [120, 40], BF16, kind="ExternalInput")
    bias = nc.dram_tensor("bias", [120, 1], F32, kind="ExternalInput")
    # Output in bf16 (halves store bytes; rel-err budget is 2e-2) and in a
    # HW-friendly scrambled layout: each half-obuf stores as one fully
    # contiguous 40x1536 run, so the store AP's outer dim is 40 partition
    # rows -> HWDGE sprays across all 16 SDMA engines instead of 5.  The
    # host unscrambles (postprocess).
    out = nc.dram_tensor("out", [BL, NSUP, 2, 40, 3 * N], BF16,
                         kind="ExternalOutput")
    out_tail = nc.dram_tensor("out_tail", [BL, 40, N], BF16,
                              kind="ExternalOutput")

    from contextlib import ExitStack
    with tile.TileContext(nc, pool_alloc_mode="queue") as tc, ExitStack() as ctx:
        singles = ctx.enter_context(tc.tile_pool(name="singles", bufs=1))
        chan_pool = ctx.enter_context(tc.tile_pool(name="chan", bufs=3))
        sq_pool = ctx.enter_context(tc.tile_pool(name="sq", bufs=30))
        obuf_pool = ctx.enter_context(tc.tile_pool(name="obuf", bufs=8))
        tmp_pool = ctx.enter_context(tc.tile_pool(name="tmp", bufs=2))
        pd_pool = ctx.enter_context(tc.tile_pool(name="pd", bufs=3, space="PSUM"))
        po_pool = ctx.enter_context(tc.tile_pool(name="po", bufs=2, space="PSUM"))

        # w1 replicated at partition bands 0/32/64/96 so four MM1s run
        # concurrently in disjoint PE row groups (tile_position packing)
        w1_t = singles.tile([126, 120], BF16)
        for r in range(4):
            nc.gpsimd.dma_start(out=w1_t[32 * r:32 * r + 30, :], in_=w1.ap())
        w2_t = singles.tile([120, 40], BF16)
        nc.gpsimd.dma_start(out=w2_t[:, :], in_=w2.ap())
        # per-partition bias s*b[k,o]: applied inside the Square activation
        # (ScalarE) / via tensor_scalar add (VectorE path)
        bias_t = singles.tile([120, 1], F32)
        nc.gpsimd.dma_start(out=bias_t[:, :], in_=bias.ap())

        def mm1(chan, band, pd, t, j):
            """Chunk j of `chan` band -> pd column t.  Bands use disjoint
            PE row groups, so the four mm1s execute concurrently."""
            nc.tensor.matmul(
                pd[:, bass.ts(t, N)], w1_t[band:band + 30, :],
                chan[band + 0:band + 30, bass.ts(j, N)],
                start=True, stop=True, tile_position=(band, 0))

        def square(pd, n_tiles):
            sq = sq_pool.tile([120, 2 * N], BF16, tag="sq")
            nc.scalar.activation(
                sq[:, 0:n_tiles * N], pd[:, 0:n_tiles * N],
                mybir.ActivationFunctionType.Square,
                bias=bias_t[:, 0:1], scale=1.0)
            return sq

        def square_dve(pd):
            """Square via VectorE: PSUM->SBUF bf16 copy (1x) + bf16
            tensor_mul (2x).  Less efficient than ScalarE but runs on
            the otherwise under-used DVE -- used to offload ScalarE."""
            tmp = tmp_pool.tile([120, 2 * N], BF16, tag="tmp")
            nc.vector.tensor_scalar_add(tmp[:, :], pd[:, 0:2 * N],
                                        bias_t[:, 0:1])
            sq = sq_pool.tile([120, 2 * N], BF16, tag="sq")
            nc.vector.tensor_mul(sq[:, :], tmp[:, :], tmp[:, :])
            return sq

        def emit_po(sq_of, i, obuf):
            """MM2 for chunks (i, i+3) -> one po bank at partition offsets
            0/64 (tile_position col packing) -> DVE cast into obuf column
            block i.  With this pairing obuf[0:40] holds chunks 0..2 and
            obuf[64:104] chunks 3..5 -- each a contiguous 1536-px bf16 run
            per (g, k)."""
            po = po_pool.tile([104, N], F32, tag="po")
            for u, j in enumerate((i, i + 3)):
                sq, col = sq_of(j)
                nc.tensor.matmul(
                    po[64 * u:64 * u + 40, :], w2_t[:, :],
                    sq[:, bass.ts(col, N)],
                    start=True, stop=True, tile_position=(0, 64 * u))
            nc.vector.tensor_copy(obuf[:, bass.ts(i, N)], po[0:104, :])

        def load_chan(chan, band, s):
            """Load super-tile s (global index) into `chan` band 0/64."""
            img, S = divmod(s, NSUP)
            ib = img * 2 * C * PLANE
            px0 = S * SPX
            src_xy = bass.AP(xy, ib + px0,
                             [[PLANE, 2 * C], [BAND, G], [1, BAND]])
            nc.gpsimd.dma_start(out=chan[band + 0:band + 30], in_=src_xy)

        def store_out(obuf, s):
            img, S = divmod(s, NSUP)
            for half in range(2):
                src = bass.AP(
                    obuf.tensor, obuf.offset + half * 64 * (3 * N),
                    [[3 * N, 40], [1, 3 * N]])
                dst = bass.AP(
                    out, ((img * NSUP + S) * 2 + half) * 40 * (3 * N),
                    [[3 * N, 40], [1, 3 * N]])
                nc.sync.dma_start(out=dst, in_=src)

        def emit_mm2_store(sqs, s0):
            for b in range(4):
                half, bi = b // 2, b % 2
                obuf = obuf_pool.tile([104, 3 * N], BF16, tag="obuf")

                def sq_of(j, half=half, bi=bi):
                    return (sqs[(half, j)], bi)
                for i in range(3):
                    emit_po(sq_of, i, obuf)
                store_out(obuf, s0 + b)

        NSUP_CORE = BL * NSUP          # 68 super-tiles per core
        pending = []
        for q in range(NSUP_CORE // 4):
            chan = chan_pool.tile([127, TPS * N], BF16, tag="chan")
            for r in range(4):
                load_chan(chan, 32 * r, 4 * q + r)

            # Drain the 2-quad-delayed MM2/store first: its inputs (squares
            # of quad q-2) are long ready, so these matmuls never stall the
            # in-order PE stream (no head-of-line blocking).
            if len(pending) == 2:
                emit_mm2_store(*pending.pop(0))

            sqs = {}
            for j in range(TPS):
                pd01 = pd_pool.tile([120, 2 * N], F32, tag="pd")
                pd23 = pd_pool.tile([120, 2 * N], F32, tag="pd")
                # all four MM1s adjacent in the PE stream -> 4-way
                # tile_position concurrency
                mm1(chan, 0, pd01, 0, j)
                mm1(chan, 32, pd01, 1, j)
                mm1(chan, 64, pd23, 0, j)
                mm1(chan, 96, pd23, 1, j)
                sqs[(0, j)] = square(pd01, 2)
                if j == 2 or (j == 5 and q % 2 == 1):
                    sqs[(1, j)] = square_dve(pd23)
                else:
                    sqs[(1, j)] = square(pd23, 2)
            pending.append((sqs, 4 * q))
        for p in pending:
            emit_mm2_store(*p)

        # image-tail overlap tiles (re-compute 1536 px each; idempotent).
        for img in range(BL):
            ib = img * 2 * C * PLANE
            chan = chan_pool.tile([95, TPS * N], BF16, tag="chan")
            src_xy = bass.AP(xy, ib + OV_BASE, [[PLANE, 2 * C], [N, G], [1, N]])
            nc.gpsimd.dma_start(out=chan[0:30, 0:N], in_=src_xy)
            obuf = obuf_pool.tile([104, 3 * N], BF16, tag="obuf")
            pd = pd_pool.tile([120, 2 * N], F32, tag="pd")
            mm1(chan, 0, pd, 0, 0)
            sq_ov = square(pd, 1)
            po = po_pool.tile([104, N], F32, tag="po")
            nc.tensor.matmul(po[0:40, :], w2_t[:, :], sq_ov[:, bass.ts(0, N)],
                             start=True, stop=True, tile_position=(0, 0))
            nc.vector.tensor_copy(obuf[:, bass.ts(0, N)], po[0:104, :])
            dst = bass.AP(out_tail, img * 40 * N, [[N, 40], [1, N]])
            src = bass.AP(obuf.tensor, obuf.offset, [[3 * N, 40], [1, N]])
            nc.sync.dma_start(out=dst, in_=src)

    return nc


def _run(in_maps, trace=False, tmpdir=None):
    nc = build_nc()
    if trace:
        # inject the NTFF profile hook (normally absent in this image)
        import sys, types
        from trn_agent_boot.trn_boot import _ntff_profile_via_ctypes
        hook = _ntff_profile_via_ctypes('/opt/axon/libaxon_pjrt.so')
        m = types.ModuleType("antenv.axon_hooks")
        m.get_axon_ntff_profile_hook = lambda: hook
        m.set_axon_ntff_profile_hook = lambda h: None
        sys.modules["antenv.axon_hooks"] = m
        bass_utils.upload_artifacts = lambda d: d
    return bass_utils.run_bass_kernel_spmd(
        nc, in_maps, core_ids=list(range(NCORES)), trace=trace, tmpdir=tmpdir)


def make_in_maps(images, shifted_images, W, b, sigma):
    w1, w2, bias = _host_weights(np.asarray(W), np.asarray(b), sigma)
    w1 = w1.astype(ml_dtypes.bfloat16)
    w2 = w2.astype(ml_dtypes.bfloat16)
    xy = np.concatenate(
        [np.asarray(images, np.float32), np.asarray(shifted_images, np.float32)],
        axis=1).astype(ml_dtypes.bfloat16)
    in_maps = []
    for i in range(NCORES):
        in_maps.append({
            "xy": np.ascontiguousarray(xy[BL * i:BL * (i + 1)]),
            "w1": w1, "w2": w2, "bias": bias,
        })
    return in_maps


def postprocess(res):
    """Unscramble the HW output layout back to [B, K, H, W] float32.

    main[img, s, h, m=(8g+k), j] -> out[img, k, s*SPX + g*BAND + h*3N + j]
    tail[img, m=(8g+k), j]       -> out[img, k, OV_BASE + g*N + j]
    """
    outs = []
    for i in range(NCORES):
        main = np.asarray(res.results[i]["out"]).astype(np.float32)
        tail = np.asarray(res.results[i]["out_tail"]).astype(np.float32)
        main = main.reshape(BL, NSUP, 2, G, K, 3 * N)
        main = main.transpose(0, 4, 1, 3, 2, 5).reshape(BL, K, NSUP * SPX)
        full = np.empty((BL, K, PLANE), np.float32)
        full[:, :, :NSUP * SPX] = main
        tail = tail.reshape(BL, G, K, N).transpose(0, 2, 1, 3)
        full[:, :, OV_BASE:] = tail.reshape(BL, K, G * N)
        outs.append(full.reshape(BL, K, H, Wd))
    return np.concatenate(outs, axis=0)


def kernel(images, shifted_images, W, b, sigma):
    in_maps = make_in_maps(images, shifted_images, W, b, sigma)
    res = _run(in_maps, trace=False)
    return postprocess(res)



# revision 20
# speedup vs baseline: 1.5231x; 1.2834x over previous
"""Trainium2 Bass kernel for nn_AffineMaskGenerator.

For each pixel p with color x (3-vec from `images`) and shifted color y
(3-vec from `shifted_images`), and each class k:

    log_mask[b, k, h, w] = -||W_k @ x + b_k - y||^2 / (2 sigma^2)

Strategy (pure data parallel over batch, 4 images per NeuronCore):
  - Fold the affine map into one matmul: with s = 1/(sqrt(2)*sigma),
    diff = s*(W_k x - y) + s*b_k is linear in (x, y); the bias enters
    later through the Square activation's per-partition bias.
    MM1: lhsT [30, 120] x rhs [30, 512] -> PSUM [120, 512]; the 120
    rows are 5 pixel-groups x (8 classes x 3 channels) = 5 px/PE-cycle.
    Four MM1s run concurrently in disjoint PE row groups 0/32/64/96
    (tile_position packing, 4 super-tiles per "quad").
  - Square via ScalarE activation(Square, bias=s*b) into bf16; ~1.5 of
    12 square tiles per quad go to VectorE (tensor_scalar add + bf16
    tensor_mul) to balance the engines.  DVE cannot read PSUM twice,
    so plain tensor_mul on PSUM is unavailable.
  - MM2: lhsT [120, 40] of -1 entries sums squares over channels ->
    [40, 512] final values in PSUM (5 groups x 8 classes).  Chunks
    (i, i+3) pack into one PSUM bank at partition offsets 0/64
    (tile_position) so a single DVE copy evacuates both, and each obuf
    half is a contiguous 1536-px run per (group, class) -> one 3-dim
    store DMA per half.
  - Matmuls run in bf16 (this walrus build rejects f32/f32r matmuls;
    PE is also clamped to ~1.2 GHz here, so tile_position concurrency
    is the only matmul-throughput lever).  Inputs are pre-cast to bf16
    on the host and concatenated to one [BL, 6, H, W] tensor: one
    3-dim SWDGE DMA loads a whole super-tile band set.
  - Pixel groups are contiguous 3072-px bands inside each 15360-px
    super-tile; the image tail is covered by an overlapping
    (idempotent) extra tile per image.
"""

import ml_dtypes
import numpy as np

import concourse.bass as bass
import concourse.mybir as mybir
import concourse.tile as tile
from concourse.tile import ScopedClock
from concourse import bass_utils

F32 = mybir.dt.float32
BF16 = mybir.dt.bfloat16

B, C, H, Wd = 32, 3, 512, 512
K = 8
NCORES = 8
BL = B // NCORES            # images per core
PLANE = H * Wd              # 262144 pixels per channel plane

N = 512                     # pixels per chunk (one PSUM bank of f32)
G = 5                       # pixel groups per tile (5 px / PE cycle)
TPS = 6                     # chunks per super-tile
BAND = TPS * N              # 3072 px: one group's contiguous band
SPX = G * BAND              # 15360 pixels per super-tile
NSUP = PLANE // SPX         # 17 full super-tiles per image
OV_BASE = PLANE - G * N     # overlap tile covers the image tail

_patched = False

# --- v5 (joint-decomposition) geometry: 16 shared squared forms, 8 groups
# per PE band, 2 bands; band = 4096 px -> supertile 65536 px, exactly 4 per
# image, no tail tiles.
R_FORMS = 16
T5 = 4096                   # px per group band
SPX5 = 16 * T5              # 65536 px per super-tile
NSUP5 = PLANE // SPX5       # 4 per image


MAX_WAITS = 1   # this walrus build rejects instructions with more sync waits


def _split_excess_waits(nc):
    """Walrus 'Too many sync wait commands': any instruction carrying
    more than MAX_WAITS sem waits gets the excess moved onto fresh NoOps
    inserted just before it on the same engine (engines execute their
    instruction stream in block order, so semantics are unchanged)."""
    import bass_rust
    counter = [0]
    for f in nc.m.functions:
        for bb in f.blocks:
            new_insts = []
            for inst in bb.instructions:
                si = inst.sync_info
                waits = list(si.on_wait or []) if si is not None else []
                if len(waits) > MAX_WAITS:
                    rest = waits[:-MAX_WAITS]
                    si.on_wait = waits[-MAX_WAITS:]
                    while rest:
                        counter[0] += 1
                        nop = bass_rust.InstNoOp(
                            name=f"waitsplit_{counter[0]}", ins=[], outs=[])
                        nop.engine = inst.engine
                        nop.sync_info = mybir.SyncInfo(
                            on_wait=rest[:MAX_WAITS], on_update=[])
                        rest = rest[MAX_WAITS:]
                        new_insts.append(nop)
                new_insts.append(inst)
            bb.instructions = new_insts


def _patch_tile_drain():
    """Rebuild the kernel-tail drain with split waits + run the global
    excess-wait splitter after Tile lowering."""
    global _patched
    if _patched:
        return
    _patched = True

    def _drain_and_barrier(self, tick_clock, wait_clock):
        drain_inst = self.nc.sync.drain()
        wait_clock.add_sem_waits(
            drain_inst.ins, ScopedClock({None: tick_clock.global_clock})
        )
        si = drain_inst.ins.sync_info
        waits = list(si.on_wait or []) if si is not None else []
        if len(waits) > 1:
            si.on_wait = waits[:1]
            for w in waits[1:]:
                d2 = self.nc.sync.drain()
                d2.ins.sync_info = mybir.SyncInfo(on_wait=[w], on_update=[])
        self.nc.all_engine_barrier()
        popped = self.nc._tile_sem_poison_stack.pop()
        assert popped is self._sem_poison
        self.nc.clear_and_free_semaphores(list(self.sems.allocated().values()))
        self.nc.all_engine_barrier()
        _split_excess_waits(self.nc)

    tile.TileContext._drain_and_barrier = _drain_and_barrier


def _decompose_forms(Wm, bm, deadline_s=360.0):
    """Find 16 affine forms m_i (in z' = (x, y, 1)) and coefficients c_ki
    with  sum_o (W_k x + b_k - y)_o^2  ~=  sum_i c_ki (m_i . z')^2 + gamma_k.

    Variable-projection Levenberg-Marquardt with an increasing penalty on
    sum_i |c_ki| s_i (cancellation/conditioning), so bf16 noise is not
    amplified.  Returns (sim_bf16_err, M [16,7], Cc [8,16], gamma [8]) of
    the best seed, or None.
    """
    import time as _time
    from scipy.optimize import least_squares

    K_, D = 8, 7
    R = R_FORMS
    A = np.zeros((K_, 3, D))
    for k in range(K_):
        A[k, :, 0:3] = Wm[k]
        A[k, :, 3:6] = -np.eye(3)
        A[k, :, 6] = bm[k]
    G = np.einsum('koa,kob->kab', A, A)
    mask = np.ones((D, D)); mask[6, 6] = 0.0
    widx = np.where(mask.reshape(-1) > 0)[0]
    gnorm = np.sqrt((G * G * mask).sum())

    mrng = np.random.default_rng(99)
    P = 256
    xz = mrng.uniform(size=(P, 3)); yz = mrng.uniform(size=(P, 3))
    Z = np.concatenate([xz, yz, np.ones((P, 1))], 1)
    d_true = np.einsum('koc,nc->nko', Wm, xz) + bm[None] - yz[:, None, :]
    F_true = (d_true ** 2).sum(-1)
    fnorm = np.linalg.norm(F_true) / np.sqrt(P * K_)
    wsq = np.sqrt(mask.reshape(-1))

    def solve_c(M):
        B = np.einsum('ra,rb->rab', M, M).reshape(R, D * D)
        Cc, *_ = np.linalg.lstsq((B * wsq).T, (G.reshape(K_, -1) * wsq).T,
                                 rcond=None)
        return Cc.T

    def fun(x, lam):
        M = x.reshape(R, D)
        Cc = solve_c(M)
        S = np.einsum('kr,ra,rb->kab', Cc, M, M)
        r1 = ((S - G).reshape(K_, -1)[:, widx]).ravel() / gnorm * 30.0
        s = (Z @ M.T) ** 2
        t = s @ np.abs(Cc).T
        return np.concatenate([r1, lam * t.ravel() / (np.sqrt(P * K_) * fnorm)])

    def bf16(v):
        return np.asarray(v, dtype=ml_dtypes.bfloat16).astype(np.float64)

    vrng = np.random.default_rng(1234)
    nv = 20000
    xv = vrng.uniform(size=(nv, 3)); yv = vrng.uniform(size=(nv, 3))
    Zv = np.concatenate([xv, yv, np.ones((nv, 1))], 1)
    dv = np.einsum('koc,nc->nko', Wm, xv) + bm[None] - yv[:, None, :]
    Fv = (dv ** 2).sum(-1)

    def sim_err(M, Cc, gamma):
        dd = bf16(Zv) @ bf16(M).T
        sq = bf16(bf16(dd) ** 2)
        Fd = sq @ bf16(Cc).T + gamma[None]
        return np.linalg.norm(Fd - Fv) / np.linalg.norm(Fv)

    gen = A.reshape(-1, D)
    t0 = _time.time()
    best = None
    for seed in range(12):
        if best is not None and (_time.time() - t0 > deadline_s
                                 or best[0] < 4.5e-3):
            break
        rng = np.random.default_rng(seed)
        idx = rng.permutation(24)[:R]
        x = (gen[idx] + 0.05 * rng.normal(size=(R, D))).ravel()
        try:
            for lam in (0.0, 0.03, 0.1):
                sol = least_squares(fun, x, args=(lam,), method='lm',
                                    xtol=1e-14, ftol=1e-14, gtol=1e-14,
                                    max_nfev=6000)
                x = sol.x
        except Exception:
            continue
        M = x.reshape(R, D)
        Cc = solve_c(M)
        S = np.einsum('kr,ra,rb->kab', Cc, M, M)
        gamma = G[:, 6, 6] - S[:, 6, 6]
        resid = np.sqrt((((S - G) * mask) ** 2).sum()) / gnorm
        if resid > 5e-3:
            continue
        e = sim_err(M, Cc, gamma)
        if best is None or e < best[0]:
            best = (e, M.copy(), Cc.copy(), gamma.copy())
    return best


def _host_weights_v5(M, Cc, gamma, sigma):
    """w1 [112,128]: band-replicated rows 64h+8c+g -> col 16g+i = s*M[i,c];
    bias [128,1] = s*M[i,6]; w2 [128,64]: row 16g+i -> col 8g+k = -c_ki;
    gbias [128,1] row 64u+8g+k = -s^2*gamma_k (added during PSUM evac)."""
    s = 1.0 / (np.sqrt(2.0) * float(sigma))
    w1 = np.zeros((112, 128), np.float32)
    bias = np.zeros((128, 1), np.float32)
    w2 = np.zeros((128, 64), np.float32)
    gb = np.zeros((128, 1), np.float32)
    for g in range(8):
        for i in range(R_FORMS):
            col = 16 * g + i
            for c in range(6):
                for h in range(2):
                    w1[64 * h + 8 * c + g, col] = s * M[i, c]
            bias[col, 0] = s * M[i, 6]
            for k in range(K):
                w2[16 * g + i, 8 * g + k] = -Cc[k, i]
    for u in range(2):
        for g in range(8):
            for k in range(K):
                gb[64 * u + 8 * g + k, 0] = -(s * s) * gamma[k]
    return w1, w2, bias, gb


def build_nc_v5():
    _patch_tile_drain()
    nc = bass.Bass("TRN2", target_bir_lowering=False, debug=False)
    xy = nc.dram_tensor("xy", [BL, 2 * C, H, Wd], BF16, kind="ExternalInput")
    w1 = nc.dram_tensor("w1", [112, 128], BF16, kind="ExternalInput")
    w2 = nc.dram_tensor("w2", [128, 64], BF16, kind="ExternalInput")
    bias = nc.dram_tensor("bias", [128, 1], F32, kind="ExternalInput")
    gbias = nc.dram_tensor("gbias", [128, 1], F32, kind="ExternalInput")
    # scrambled bf16 output: [img, S, band, chunk-parity, m=(8g+k), 4*512]
    out = nc.dram_tensor("out", [BL, NSUP5, 2, 2, 64, 4 * N], BF16,
                         kind="ExternalOutput")

    from contextlib import ExitStack
    with tile.TileContext(nc, pool_alloc_mode="queue") as tc, ExitStack() as ctx:
        singles = ctx.enter_context(tc.tile_pool(name="singles", bufs=1))
        chan_pool = ctx.enter_context(tc.tile_pool(name="chan", bufs=3))
        sq_pool = ctx.enter_context(tc.tile_pool(name="sq", bufs=28))
        obuf_pool = ctx.enter_context(tc.tile_pool(name="obuf", bufs=6))
        tmp_pool = ctx.enter_context(tc.tile_pool(name="tmp", bufs=2))
        pd_pool = ctx.enter_context(tc.tile_pool(name="pd", bufs=3, space="PSUM"))
        po_pool = ctx.enter_context(tc.tile_pool(name="po", bufs=2, space="PSUM"))

        w1_t = singles.tile([112, 128], BF16)
        nc.gpsimd.dma_start(out=w1_t[:, :], in_=w1.ap())
        w2_t = singles.tile([128, 64], BF16)
        nc.gpsimd.dma_start(out=w2_t[:, :], in_=w2.ap())
        bias_t = singles.tile([128, 1], F32)
        nc.gpsimd.dma_start(out=bias_t[:, :], in_=bias.ap())
        gbias_t = singles.tile([128, 1], F32)
        nc.gpsimd.dma_start(out=gbias_t[:, :], in_=gbias.ap())

        def square_v5(pd, act):
            sq = sq_pool.tile([128, 2 * N], BF16, tag="sq")
            if act:
                nc.scalar.activation(
                    sq[:, :], pd[:, 0:2 * N],
                    mybir.ActivationFunctionType.Square,
                    bias=bias_t[:, 0:1], scale=1.0)
            else:
                tmp = tmp_pool.tile([128, 2 * N], BF16, tag="tmp")
                nc.vector.tensor_scalar_add(tmp[:, :], pd[:, 0:2 * N],
                                            bias_t[:, 0:1])
                nc.vector.tensor_mul(sq[:, :], tmp[:, :], tmp[:, :])
            return sq

        def emit_mm2_store_v5(sqs, img, S):
            for h in range(2):
                obuf = obuf_pool.tile([128, 4 * N], BF16, tag="obuf")
                for t in range(4):
                    po = po_pool.tile([128, N], F32, tag="po")
                    sq = sqs[(h, t)]
                    for u in range(2):
                        nc.tensor.matmul(
                            po[64 * u:64 * u + 64, :], w2_t[:, :],
                            sq[:, bass.ts(u, N)],
                            start=True, stop=True, tile_position=(0, 64 * u))
                    nc.vector.tensor_scalar_add(
                        obuf[:, bass.ts(t, N)], po[0:128, :], gbias_t[:, 0:1])
                for u in range(2):
                    base = (((img * NSUP5 + S) * 2 + h) * 2 + u) * 64 * (4 * N)
                    src = bass.AP(obuf.tensor, obuf.offset + u * 64 * (4 * N),
                                  [[4 * N, 64], [1, 4 * N]])
                    dst = bass.AP(out, base, [[4 * N, 64], [1, 4 * N]])
                    nc.sync.dma_start(out=dst, in_=src)

        pending = []
        for idx in range(BL * NSUP5):
            img, S = divmod(idx, NSUP5)
            chan = chan_pool.tile([112, T5], BF16, tag="chan")
            for h in range(2):
                src = bass.AP(xy, img * 2 * C * PLANE + S * SPX5 + h * 8 * T5,
                              [[PLANE, 2 * C], [T5, 8], [1, T5]])
                nc.gpsimd.dma_start(out=chan[64 * h:64 * h + 48, :], in_=src)
            # stall-free PE work first (inputs 2 super-tiles old)
            if len(pending) == 2:
                emit_mm2_store_v5(*pending.pop(0))
            sqs = {}
            for t in range(4):
                pdA = pd_pool.tile([128, 2 * N], F32, tag="pd")
                pdB = pd_pool.tile([128, 2 * N], F32, tag="pd")
                for u in range(2):
                    c = 2 * t + u
                    nc.tensor.matmul(
                        pdA[:, bass.ts(u, N)], w1_t[0:48, :],
                        chan[0:48, bass.ts(c, N)],
                        start=True, stop=True, tile_position=(0, 0))
                    nc.tensor.matmul(
                        pdB[:, bass.ts(u, N)], w1_t[64:112, :],
                        chan[64:112, bass.ts(c, N)],
                        start=True, stop=True, tile_position=(64, 0))
                sqs[(0, t)] = square_v5(pdA, act=True)
                sqs[(1, t)] = square_v5(pdB, act=(t != 1))
            pending.append((sqs, img, S))
        for p in pending:
            emit_mm2_store_v5(*p)

    return nc


def make_in_maps_v5(images, shifted_images, M, Cc, gamma, sigma):
    w1, w2, bias, gb = _host_weights_v5(M, Cc, gamma, sigma)
    w1 = w1.astype(ml_dtypes.bfloat16)
    w2 = w2.astype(ml_dtypes.bfloat16)
    xy = np.concatenate(
        [np.asarray(images, np.float32), np.asarray(shifted_images, np.float32)],
        axis=1).astype(ml_dtypes.bfloat16)
    in_maps = []
    for i in range(NCORES):
        in_maps.append({
            "xy": np.ascontiguousarray(xy[BL * i:BL * (i + 1)]),
            "w1": w1, "w2": w2, "bias": bias, "gbias": gb,
        })
    return in_maps


def postprocess_v5(res):
    """buf[img, S, h, u, 8g+k, 512t+e] -> out[img, k, S*65536 + h*32768 +
    g*4096 + t*1024 + u*512 + e]"""
    outs = []
    for i in range(NCORES):
        buf = np.asarray(res.results[i]["out"]).astype(np.float32)
        buf = buf.reshape(BL, NSUP5, 2, 2, 8, K, 4, N)
        buf = buf.transpose(0, 5, 1, 2, 4, 6, 3, 7).reshape(BL, K, PLANE)
        outs.append(buf.reshape(BL, K, H, Wd))
    return np.concatenate(outs, axis=0)


def _host_weights(Wm, bm, sigma):
    """w1 [31, 120]: row 5c+g = x_c of group g, 15+5o+g = y_o of group g,
    30 = ones; col m = 24g+3k+o.  w2 [120, 40]: channel-sum, col 8g+k."""
    s = 1.0 / (np.sqrt(2.0) * float(sigma))
    w1 = np.zeros((30, 120), np.float32)
    w2 = np.zeros((120, 40), np.float32)
    bias = np.zeros((120, 1), np.float32)
    for g in range(G):
        for k in range(K):
            for o in range(C):
                m = 24 * g + 3 * k + o
                for c in range(C):
                    w1[5 * c + g, m] = s * Wm[k, o, c]
                w1[15 + 5 * o + g, m] = -s
                bias[m, 0] = s * bm[k, o]
                w2[m, 8 * g + k] = -1.0
    return w1, w2, bias


def build_nc():
    _patch_tile_drain()
    nc = bass.Bass("TRN2", target_bir_lowering=False, debug=False)
    # xy: host-side concat of images & shifted along channels, pre-cast
    # to bf16 (halves input DMA bytes, avoids the slow SWDGE cast path)
    xy = nc.dram_tensor("xy", [BL, 2 * C, H, Wd], BF16, kind="ExternalInput")
    w1 = nc.dram_tensor("w1", [30, 120], BF16, kind="ExternalInput")
    w2 = nc.dram_tensor("w2", [120, 40], BF16, kind="ExternalInput")
    bias = nc.dram_tensor("bias", [120, 1], F32, kind="ExternalInput")
    # Output in bf16 (halves store bytes; rel-err budget is 2e-2) and in a
    # HW-friendly scrambled layout: each half-obuf stores as one fully
    # contiguous 40x1536 run, so the store AP's outer dim is 40 partition
    # rows -> HWDGE sprays across all 16 SDMA engines instead of 5.  The
    # host unscrambles (postprocess).
    out = nc.dram_tensor("out", [BL, NSUP, 2, 40, 3 * N], BF16,
                         kind="ExternalOutput")
    out_tail = nc.dram_tensor("out_tail", [BL, 40, N], BF16,
                              kind="ExternalOutput")

    from contextlib import ExitStack
    with tile.TileContext(nc, pool_alloc_mode="queue") as tc, ExitStack() as ctx:
        singles = ctx.enter_context(tc.tile_pool(name="singles", bufs=1))
        chan_pool = ctx.enter_context(tc.tile_pool(name="chan", bufs=3))
        sq_pool = ctx.enter_context(tc.tile_pool(name="sq", bufs=30))
        obuf_pool = ctx.enter_context(tc.tile_pool(name="obuf", bufs=8))
        tmp_pool = ctx.enter_context(tc.tile_pool(name="tmp", bufs=2))
        pd_pool = ctx.enter_context(tc.tile_pool(name="pd", bufs=3, space="PSUM"))
        po_pool = ctx.enter_context(tc.tile_pool(name="po", bufs=2, space="PSUM"))

        # w1 replicated at partition bands 0/32/64/96 so four MM1s run
        # concurrently in disjoint PE row groups (tile_position packing)
        w1_t = singles.tile([126, 120], BF16)
        for r in range(4):
            nc.gpsimd.dma_start(out=w1_t[32 * r:32 * r + 30, :], in_=w1.ap())
        w2_t = singles.tile([120, 40], BF16)
        nc.gpsimd.dma_start(out=w2_t[:, :], in_=w2.ap())
        # per-partition bias s*b[k,o]: applied inside the Square activation
        # (ScalarE) / via tensor_scalar add (VectorE path)
        bias_t = singles.tile([120, 1], F32)
        nc.gpsimd.dma_start(out=bias_t[:, :], in_=bias.ap())

        def mm1(chan, band, pd, t, j):
            """Chunk j of `chan` band -> pd column t.  Bands use disjoint
            PE row groups, so the four mm1s execute concurrently."""
            nc.tensor.matmul(
                pd[:, bass.ts(t, N)], w1_t[band:band + 30, :],
                chan[band + 0:band + 30, bass.ts(j, N)],
                start=True, stop=True, tile_position=(band, 0))

        def square(pd, n_tiles):
            sq = sq_pool.tile([120, 2 * N], BF16, tag="sq")
            nc.scalar.activation(
                sq[:, 0:n_tiles * N], pd[:, 0:n_tiles * N],
                mybir.ActivationFunctionType.Square,
                bias=bias_t[:, 0:1], scale=1.0)
            return sq

        def square_dve(pd):
            """Square via VectorE: PSUM->SBUF bf16 copy (1x) + bf16
            tensor_mul (2x).  Less efficient than ScalarE but runs on
            the otherwise under-used DVE -- used to offload ScalarE."""
            tmp = tmp_pool.tile([120, 2 * N], BF16, tag="tmp")
            nc.vector.tensor_scalar_add(tmp[:, :], pd[:, 0:2 * N],
                                        bias_t[:, 0:1])
            sq = sq_pool.tile([120, 2 * N], BF16, tag="sq")
            nc.vector.tensor_mul(sq[:, :], tmp[:, :], tmp[:, :])
            return sq

        def emit_po(sq_of, i, obuf):
            """MM2 for chunks (i, i+3) -> one po bank at partition offsets
            0/64 (tile_position col packing) -> DVE cast into obuf column
            block i.  With this pairing obuf[0:40] holds chunks 0..2 and
            obuf[64:104] chunks 3..5 -- each a contiguous 1536-px bf16 run
            per (g, k)."""
            po = po_pool.tile([104, N], F32, tag="po")
            for u, j in enumerate((i, i + 3)):
                sq, col = sq_of(j)
                nc.tensor.matmul(
                    po[64 * u:64 * u + 40, :], w2_t[:, :],
                    sq[:, bass.ts(col, N)],
                    start=True, stop=True, tile_position=(0, 64 * u))
            nc.vector.tensor_copy(obuf[:, bass.ts(i, N)], po[0:104, :])

        def load_chan(chan, band, s):
            """Load super-tile s (global index) into `chan` band 0/64."""
            img, S = divmod(s, NSUP)
            ib = img * 2 * C * PLANE
            px0 = S * SPX
            src_xy = bass.AP(xy, ib + px0,
                             [[PLANE, 2 * C], [BAND, G], [1, BAND]])
            nc.gpsimd.dma_start(out=chan[band + 0:band + 30], in_=src_xy)

        def store_out(obuf, s):
            img, S = divmod(s, NSUP)
            for half in range(2):
                src = bass.AP(
                    obuf.tensor, obuf.offset + half * 64 * (3 * N),
                    [[3 * N, 40], [1, 3 * N]])
                dst = bass.AP(
                    out, ((img * NSUP + S) * 2 + half) * 40 * (3 * N),
                    [[3 * N, 40], [1, 3 * N]])
                nc.sync.dma_start(out=dst, in_=src)

        def emit_mm2_store(sqs, s0):
            for b in range(4):
                half, bi = b // 2, b % 2
                obuf = obuf_pool.tile([104, 3 * N], BF16, tag="obuf")

                def sq_of(j, half=half, bi=bi):
                    return (sqs[(half, j)], bi)
                for i in range(3):
                    emit_po(sq_of, i, obuf)
                store_out(obuf, s0 + b)

        NSUP_CORE = BL * NSUP          # 68 super-tiles per core
        pending = []
        for q in range(NSUP_CORE // 4):
            chan = chan_pool.tile([127, TPS * N], BF16, tag="chan")
            for r in range(4):
                load_chan(chan, 32 * r, 4 * q + r)

            # Drain the 2-quad-delayed MM2/store first: its inputs (squares
            # of quad q-2) are long ready, so these matmuls never stall the
            # in-order PE stream (no head-of-line blocking).
            if len(pending) == 2:
                emit_mm2_store(*pending.pop(0))

            sqs = {}
            for j in range(TPS):
                pd01 = pd_pool.tile([120, 2 * N], F32, tag="pd")
                pd23 = pd_pool.tile([120, 2 * N], F32, tag="pd")
                # all four MM1s adjacent in the PE stream -> 4-way
                # tile_position concurrency
                mm1(chan, 0, pd01, 0, j)
                mm1(chan, 32, pd01, 1, j)
                mm1(chan, 64, pd23, 0, j)
                mm1(chan, 96, pd23, 1, j)
                sqs[(0, j)] = square(pd01, 2)
                if j == 2 or (j == 5 and q % 2 == 1):
                    sqs[(1, j)] = square_dve(pd23)
                else:
                    sqs[(1, j)] = square(pd23, 2)
            pending.append((sqs, 4 * q))
        for p in pending:
            emit_mm2_store(*p)

        # image-tail overlap tiles (re-compute 1536 px each; idempotent).
        for img in range(BL):
            ib = img * 2 * C * PLANE
            chan = chan_pool.tile([95, TPS * N], BF16, tag="chan")
            src_xy = bass.AP(xy, ib + OV_BASE, [[PLANE, 2 * C], [N, G], [1, N]])
            nc.gpsimd.dma_start(out=chan[0:30, 0:N], in_=src_xy)
            obuf = obuf_pool.tile([104, 3 * N], BF16, tag="obuf")
            pd = pd_pool.tile([120, 2 * N], F32, tag="pd")
            mm1(chan, 0, pd, 0, 0)
            sq_ov = square(pd, 1)
            po = po_pool.tile([104, N], F32, tag="po")
            nc.tensor.matmul(po[0:40, :], w2_t[:, :], sq_ov[:, bass.ts(0, N)],
                             start=True, stop=True, tile_position=(0, 0))
            nc.vector.tensor_copy(obuf[:, bass.ts(0, N)], po[0:104, :])
            dst = bass.AP(out_tail, img * 40 * N, [[N, 40], [1, N]])
            src = bass.AP(obuf.tensor, obuf.offset, [[3 * N, 40], [1, N]])
            nc.sync.dma_start(out=dst, in_=src)

    return nc


def prepare(images, shifted_images, W, b, sigma):
    """Choose kernel variant: v5 (16 joint squared forms) when the
    decomposition is numerically good, else the v3b per-class path.
    Returns (build_fn, in_maps, postprocess_fn)."""
    decomp = None
    try:
        decomp = _decompose_forms(np.asarray(W, np.float64),
                                  np.asarray(b, np.float64))
    except Exception:
        decomp = None
    if decomp is not None and decomp[0] < 9e-3:
        _, M, Cc, gamma = decomp
        in_maps = make_in_maps_v5(images, shifted_images, M, Cc, gamma, sigma)
        return build_nc_v5, in_maps, postprocess_v5
    in_maps = make_in_maps(images, shifted_images, W, b, sigma)
    return build_nc, in_maps, postprocess


def _run(in_maps, trace=False, tmpdir=None, build_fn=None):
    nc = (build_fn or build_nc)()
    if trace:
        # inject the NTFF profile hook (normally absent in this image)
        import sys, types
        from trn_agent_boot.trn_boot import _ntff_profile_via_ctypes
        hook = _ntff_profile_via_ctypes('/opt/axon/libaxon_pjrt.so')
        m = types.ModuleType("antenv.axon_hooks")
        m.get_axon_ntff_profile_hook = lambda: hook
        m.set_axon_ntff_profile_hook = lambda h: None
        sys.modules["antenv.axon_hooks"] = m
        bass_utils.upload_artifacts = lambda d: d
    return bass_utils.run_bass_kernel_spmd(
        nc, in_maps, core_ids=list(range(NCORES)), trace=trace, tmpdir=tmpdir)


def make_in_maps(images, shifted_images, W, b, sigma):
    w1, w2, bias = _host_weights(np.asarray(W), np.asarray(b), sigma)
    w1 = w1.astype(ml_dtypes.bfloat16)
    w2 = w2.astype(ml_dtypes.bfloat16)
    xy = np.concatenate(
        [np.asarray(images, np.float32), np.asarray(shifted_images, np.float32)],
        axis=1).astype(ml_dtypes.bfloat16)
    in_maps = []
    for i in range(NCORES):
        in_maps.append({
            "xy": np.ascontiguousarray(xy[BL * i:BL * (i + 1)]),
            "w1": w1, "w2": w2, "bias": bias,
        })
    return in_maps


def postprocess(res):
    """Unscramble the HW output layout back to [B, K, H, W] float32.

    main[img, s, h, m=(8g+k), j] -> out[img, k, s*SPX + g*BAND + h*3N + j]
    tail[img, m=(8g+k), j]       -> out[img, k, OV_BASE + g*N + j]
    """
    outs = []
    for i in range(NCORES):
        main = np.asarray(res.results[i]["out"]).astype(np.float32)
        tail = np.asarray(res.results[i]["out_tail"]).astype(np.float32)
        main = main.reshape(BL, NSUP, 2, G, K, 3 * N)
        main = main.transpose(0, 4, 1, 3, 2, 5).reshape(BL, K, NSUP * SPX)
        full = np.empty((BL, K, PLANE), np.float32)
        full[:, :, :NSUP * SPX] = main
        tail = tail.reshape(BL, G, K, N).transpose(0, 2, 1, 3)
        full[:, :, OV_BASE:] = tail.reshape(BL, K, G * N)
        outs.append(full.reshape(BL, K, H, Wd))
    return np.concatenate(outs, axis=0)


def kernel(images, shifted_images, W, b, sigma):
    build_fn, in_maps, post = prepare(images, shifted_images, W, b, sigma)
    res = _run(in_maps, trace=False, build_fn=build_fn)
    return post(res)

